# revision 29
# baseline (speedup 1.0000x reference)
"""Trainium2 Bass kernel for AgreementRouting (dynamic routing / capsule-style).

Full-input contract: kernel(u_predict[64,2048,32,16] f32, b[2048,32] f32) -> v[64,32,16] f32.
Internally shards batch (64) across 8 NeuronCores (8 batch elems per core).

Per-core algorithm (B_loc=8, L=2048, H=32, D=16, HD=512), fp16 compute with
fp32 accumulation; batch elements processed in two resident groups of 4 with
phase-staggered emission so PE matmul bursts from different batch elements
interleave (keeps the PE HAM-warm and batches same-LUT ACT ops):
  load: u fp32 HBM -> SBUF fp16 via gpsimd casting DMA (16 tiles [128 l, 512 hd]/b)
  u^T:  PE transpose-mode matmuls (4 per PSUM bank) + DVE bitcast-u32 evac
        -> 4 tiles [128 hd, 2048 l] fp16 per b
  init: c0 = softmax(b) (shared); then per routing iteration:
    agreement: upd[l,h] = sum_hd u^T[hd,l]*V[hd,h], u^T slices as FWL fp16
               weights, block-diag V_mat [128,32] as rhs; += into b_batch fp32
    softmax:   half-width-split add/exp/reduce/recip/mult chain (ACT+DVE)
    ws:        O2[hd,h'] = sum_l u[l,hd]*c[l,h'] with u slices as weights;
               s extracted via DVE mask-multiply + inner-reduce (no PSUM round
               trip); squash factor via tiny constant matmuls (h_mat/ind_t)
               entirely in [hd-partition] layout; V_mat built by broadcast-mult
               with ind_mask
  output: final v^T [128,4] fp32 DMA'd per batch elem
"""

import sys
import os

sys.path.insert(0, "/opt/trn_rl_repo")

import numpy as np
from contextlib import ExitStack

B, L, H, D = 64, 2048, 32, 16
NCORES = 8
BLOC = B // NCORES  # 8
HD = H * D  # 512
NT = L // 128  # 16 l-chunks
NKC = HD // 128  # 4 hd chunks
NITER = 3
EPS = 1e-8

_NC_CACHE = {}
LAST_EXEC_NS = None
LAST_RESULTS = None
LAST_TRACE_DIR = None
_TRACE = False


def _consts():
    p = np.arange(128)
    j = np.arange(HD)
    # mask_hd[h', hd] = 1 iff h' == hd//16
    mask_hd = (j[None, :] // D == np.arange(H)[:, None]).astype(np.float32)
    # ind_mask[p, H*c + h] = 1 iff h == 8c + p//16   (h-index of hd = 128c + p)
    ind_mask = np.zeros((128, NKC * H), np.float32)
    for c in range(NKC):
        ind_mask[p, H * c + 8 * c + p // 16] = 1.0
    # h_mat[p, g] = 1 iff p//16 == g
    h_mat = (p[:, None] // 16 == np.arange(8)[None, :]).astype(np.float32)
    ind_t = np.ascontiguousarray(h_mat.T)  # [8, 128]
    ident16 = np.eye(128, dtype=np.float16)
    return {
        "ind_mask": ind_mask,
        "h16": h_mat,
        "it16": ind_t,
        "ident16": ident16,
    }


def _emit(ctx, tc, t_in, t_out):
    import concourse.mybir as mybir

    nc = tc.nc
    f32 = mybir.dt.float32
    f16 = mybir.dt.float16
    AF = mybir.ActivationFunctionType
    ALU = mybir.AluOpType
    AX = mybir.AxisListType

    u_ap = t_in["u"]
    b_ap = t_in["b"]
    im_ap = t_in["ind_mask"]
    h_ap = t_in["h16"]
    it_ap = t_in["it16"]
    id_ap = t_in["ident16"]
    vout_ap = t_out["v_out"]

    GRP = 4  # batch elems per resident group

    cpool = ctx.enter_context(tc.tile_pool(name="cpool", bufs=1))
    p_unat = ctx.enter_context(tc.tile_pool(name="p_unat", bufs=(GRP + 1) * NT))
    p_uT = ctx.enter_context(tc.tile_pool(name="p_uT", bufs=(GRP + 1) * NKC))
    p_bb = ctx.enter_context(tc.tile_pool(name="p_bb", bufs=BLOC))
    p_soft = ctx.enter_context(tc.tile_pool(name="p_soft", bufs=6))
    p_small = ctx.enter_context(tc.tile_pool(name="p_small", bufs=10))
    p_prod = ctx.enter_context(tc.tile_pool(name="p_prod", bufs=4))
    p_s4 = ctx.enter_context(tc.tile_pool(name="p_s4", bufs=12))
    p_sq = ctx.enter_context(tc.tile_pool(name="p_sq", bufs=24))
    p_vm = ctx.enter_context(tc.tile_pool(name="p_vm", bufs=2 * GRP * NKC))
    ps_upd = ctx.enter_context(tc.tile_pool(name="ps_upd", bufs=2, space="PSUM"))
    ps_o = ctx.enter_context(tc.tile_pool(name="ps_o", bufs=2, space="PSUM"))
    ps_t = ctx.enter_context(tc.tile_pool(name="ps_t", bufs=2, space="PSUM"))
    ps_tr = ctx.enter_context(tc.tile_pool(name="ps_tr", bufs=2, space="PSUM"))

    # ---- constants
    im_t = cpool.tile([128, NKC * H], f32, name="im_t")
    nc.sync.dma_start(im_t[:], im_ap)
    h_t = cpool.tile([128, 8], f32, name="h_t")
    nc.sync.dma_start(h_t[:], h_ap)
    it_t = cpool.tile([8, 128], f32, name="it_t")
    nc.sync.dma_start(it_t[:], it_ap)
    id_t = cpool.tile([128, 128], f16, name="id_t")
    nc.sync.dma_start(id_t[:], id_ap)
    bin_t = cpool.tile([128, NT * H], f32, name="bin_t")
    nc.sync.dma_start(
        bin_t[:].rearrange("p (t h) -> p t h", t=NT),
        b_ap.rearrange("(t p) h -> p t h", p=128),
    )

    # ---- c0 = softmax(b) over h (shared across batch; logits bounded, so no
    # max-subtraction needed)
    e0 = p_soft.tile([128, NT * H], f32, name="e0", tag="soft")
    nc.scalar.activation(e0[:], bin_t[:], AF.Exp)
    z0 = p_small.tile([128, NT], f32, name="z0", tag="small")
    nc.vector.reduce_sum(z0[:], e0[:].rearrange("p (t h) -> p t h", t=NT), AX.X)
    r0 = p_small.tile([128, NT], f32, name="r0", tag="small")
    nc.vector.reciprocal(r0[:], z0[:])
    c0 = cpool.tile([128, NT * H], f16, name="c0")
    nc.vector.tensor_tensor(
        c0[:].rearrange("p (t h) -> p t h", t=NT),
        e0[:].rearrange("p (t h) -> p t h", t=NT),
        r0[:].unsqueeze(2).broadcast_to((128, NT, H)),
        ALU.mult,
    )

    st = {}  # per-b state

    def emit_prep(b):
        nat = []
        for t in range(NT):
            s16 = p_unat.tile([128, HD], f16, name="s16", tag="unat")
            nc.gpsimd.dma_start(
                s16[:],
                u_ap[b, 128 * t : 128 * (t + 1)].rearrange("l h d -> l (h d)"),
            )
            nat.append(s16)
        st[b] = {"nat": nat}

    def emit_transpose(b):
        nat = st[b]["nat"]
        uT = []
        for k in range(NKC):
            uTk = p_uT.tile([128, L], f16, name="uTk", tag="uT")
            uT.append(uTk)
        for k in range(NKC):
            for tq in range(NT // 4):
                ptr = ps_tr.tile([128, 4 * 128], f16, name="ptr", tag="ptr", padded_shape=[128, 1024])
                for j in range(4):
                    nc.tensor.transpose(
                        ptr[:, 128 * j : 128 * (j + 1)],
                        nat[4 * tq + j][:, 128 * k : 128 * (k + 1)],
                        id_t[:],
                    )
                dst = uT[k][:, 512 * tq : 512 * (tq + 1)]
                u32 = mybir.dt.uint32
                nc.vector.tensor_copy(dst.bitcast(u32), ptr[:].bitcast(u32))
        st[b]["uT"] = uT
        bb_t = p_bb.tile([128, NT * H], f32, name="bbt", tag="bb")
        nc.vector.tensor_copy(bb_t[:], bin_t[:])
        st[b]["bb"] = bb_t

    def emit_ws(b, c_tile, last):
        """weighted-sum via u-as-weights: O2[hd, h'] = sum_l u[l, hd] c[l, h'],
        then fused mask-multiply-reduce extracts s directly into SBUF."""
        nat = st[b]["nat"]
        O2 = ps_o.tile([128, NKC * H], f32, name="O2", tag="O", padded_shape=[128, 512])
        cv = c_tile[:].rearrange("p (t h) -> p t h", t=NT)
        for k in range(NKC):
            for t in range(NT):
                nc.tensor.matmul(
                    O2[:, H * k : H * (k + 1)],
                    nat[t][:, 128 * k : 128 * (k + 1)],
                    cv[:, t, :],
                    start=(t == 0),
                    stop=(t == NT - 1),
                )
        prod = p_prod.tile([128, NKC * H], f32, name="prod", tag="prod")
        s_sb = p_s4.tile([128, NKC], f32, name="s_sb", tag="s4")
        nc.vector.tensor_tensor(prod[:], O2[:], im_t[:], ALU.mult)
        nc.vector.reduce_sum(
            s_sb[:], prod[:].rearrange("p (k h) -> p k h", k=NKC), AX.X
        )
        s2 = p_s4.tile([128, NKC], f32, name="s2", tag="s4")
        nc.scalar.square(s2[:], s_sb[:])
        # sq^T[g, c] = ||s_h||^2 for h = 8c + g
        sqT = ps_t.tile([8, NKC], f32, name="sqT", tag="pt", padded_shape=[128, 512])
        nc.tensor.matmul(sqT[:], h_t[:], s2[:], start=True, stop=True)
        st[b]["s_sb"] = s_sb
        st[b]["sqT"] = sqT
        st[b]["last"] = last

    def emit_squash(b):
        """squash factor f = sq/(1+sq)/sqrt(sq+eps); vT; V_mat (or output DMA)."""
        sqT = st[b]["sqT"]
        s_sb = st[b]["s_sb"]
        last = st[b]["last"]
        t1 = p_sq.tile([8, NKC], f32, name="t1", tag="sq")
        nc.vector.tensor_scalar_add(t1[:], sqT[:], 1.0)
        r1 = p_sq.tile([8, NKC], f32, name="r1", tag="sq")
        nc.vector.reciprocal(r1[:], t1[:])
        teps = p_sq.tile([8, NKC], f32, name="teps", tag="sq")
        nc.vector.tensor_scalar_add(teps[:], sqT[:], EPS)
        rt = p_sq.tile([8, NKC], f32, name="rt", tag="sq")
        nc.scalar.activation(rt[:], teps[:], AF.Sqrt)
        r2 = p_sq.tile([8, NKC], f32, name="r2", tag="sq")
        nc.vector.reciprocal(r2[:], rt[:])
        g1 = p_sq.tile([8, NKC], f32, name="g1", tag="sq")
        nc.vector.tensor_tensor(g1[:], sqT[:], r1[:], ALU.mult)
        fT = p_sq.tile([8, NKC], f32, name="fT", tag="sq")
        nc.vector.tensor_tensor(fT[:], g1[:], r2[:], ALU.mult)
        # expand f to hd-partition layout: fexp[p, c] = f[8c + p//16]
        fexp = ps_t.tile([128, NKC], f32, name="fexp", tag="pt", padded_shape=[128, 512])
        nc.tensor.matmul(fexp[:], it_t[:], fT[:], start=True, stop=True)
        vT = p_s4.tile([128, NKC], f32, name="vT", tag="s4")
        nc.vector.tensor_tensor(vT[:], s_sb[:], fexp[:], ALU.mult)
        if last:
            nc.sync.dma_start(
                vout_ap[b].rearrange("h d -> (h d)").rearrange("(c p) -> p c", p=128),
                vT[:],
            )
            st[b]["vms"] = None
            return
        vms = []
        for c in range(NKC):
            vm_c = p_vm.tile([128, H], f16, name="vmc", tag="vm")
            nc.vector.tensor_tensor(
                vm_c[:],
                vT[:, c : c + 1].broadcast_to((128, H)),
                im_t[:, H * c : H * (c + 1)],
                ALU.mult,
            )
            vms.append(vm_c)
        st[b]["vms"] = vms

    def emit_agree(b):
        """agreement matmuls + b_batch add + softmax -> fresh c tile."""
        uT = st[b]["uT"]
        vms = st[b]["vms"]
        bb_t = st[b]["bb"]
        upd = ps_upd.tile([128, NT * H], f32, name="upd", tag="upd", padded_shape=[128, 512])
        for t in range(NT):
            for k in range(NKC):
                nc.tensor.matmul(
                    upd[:, H * t : H * (t + 1)],
                    uT[k][:, 128 * t : 128 * (t + 1)],
                    vms[k][:],
                    start=(k == 0),
                    stop=(k == NKC - 1),
                )
        HW2 = NT * H // 2
        for hh in range(2):
            sl = slice(hh * HW2, (hh + 1) * HW2)
            nc.vector.tensor_tensor(bb_t[:, sl], bb_t[:, sl], upd[:, sl], ALU.add)

    def emit_softmax(b):
        # half-width split: pipeline the add/exp/reduce/mult chain to cut the
        # exposed latency before ws can start
        bb_t = st[b]["bb"]
        HW2 = NT * H // 2
        e = p_soft.tile([128, NT * H], f32, name="e", tag="soft")
        z = p_small.tile([128, NT], f32, name="z", tag="small")
        r = p_small.tile([128, NT], f32, name="r", tag="small")
        c_t = p_soft.tile([128, NT * H], f16, name="ct", tag="softc")
        for hh in range(2):
            sl = slice(hh * HW2, (hh + 1) * HW2)
            slz = slice(hh * NT // 2, (hh + 1) * NT // 2)
            nc.scalar.activation(e[:, sl], bb_t[:, sl], AF.Exp)
            nc.vector.reduce_sum(
                z[:, slz],
                e[:, sl].rearrange("p (t h) -> p t h", t=NT // 2),
                AX.X,
            )
            nc.vector.reciprocal(r[:, slz], z[:, slz])
            nc.vector.tensor_tensor(
                c_t[:, sl].rearrange("p (t h) -> p t h", t=NT // 2),
                e[:, sl].rearrange("p (t h) -> p t h", t=NT // 2),
                r[:, slz].unsqueeze(2).broadcast_to((128, NT // 2, H)),
                ALU.mult,
            )
        st[b]["c"] = c_t

    for g in range(BLOC // GRP):
        bs = list(range(g * GRP, (g + 1) * GRP))
        for b in bs:
            emit_prep(b)
        for b in bs:
            emit_transpose(b)
        # init weighted-sum pass with shared c0
        for b in bs:
            emit_ws(b, c0, False)
        for b in bs:
            emit_squash(b)
        for it in range(NITER):
            last = it == NITER - 1
            # staggered: alternate LDW-heavy agree bursts with MM-heavy ws bursts
            emit_agree(bs[0])
            emit_agree(bs[1])
            for j in range(GRP):
                emit_softmax(bs[j])
                if j + 2 < GRP:
                    emit_agree(bs[j + 2])
                emit_ws(bs[j], st[bs[j]]["c"], last)
            for b in bs:
                emit_squash(b)


def _get_nc():
    if "nc" in _NC_CACHE:
        return _NC_CACHE["nc"]
    from concourse import bacc
    import concourse.tile as tile
    import concourse.mybir as mybir

    f32 = mybir.dt.float32
    f16 = mybir.dt.float16
    nc = bacc.Bacc("TRN2", target_bir_lowering=False, debug=False)
    t_in = {}
    in_shapes = {
        "u": ([BLOC, L, H, D], f32),
        "b": ([L, H], f32),
        "ind_mask": ([128, NKC * H], f32),
        "h16": ([128, 8], f32),
        "it16": ([8, 128], f32),
        "ident16": ([128, 128], f16),
    }
    for name, (shape, dt_) in in_shapes.items():
        t_in[name] = nc.dram_tensor(name, shape, dt_, kind="ExternalInput").ap()
    vout = nc.dram_tensor("v_out", [BLOC, H, D], f32, kind="ExternalOutput").ap()

    with tile.TileContext(nc) as tc:
        with ExitStack() as ctx:
            _emit(ctx, tc, t_in, {"v_out": vout})
    nc.compile()
    _NC_CACHE["nc"] = nc
    return nc


def kernel(u_predict, b):
    global LAST_EXEC_NS, LAST_RESULTS
    u = np.ascontiguousarray(np.asarray(u_predict, dtype=np.float32))
    bq = np.ascontiguousarray(np.asarray(b, dtype=np.float32))
    assert u.shape == (B, L, H, D), u.shape
    assert bq.shape == (L, H), bq.shape

    nc = _get_nc()
    consts = _consts()
    in_maps = []
    for i in range(NCORES):
        m = {"u": np.ascontiguousarray(u[i * BLOC : (i + 1) * BLOC]), "b": bq}
        m.update(consts)
        in_maps.append(m)

    from concourse.bass_utils import run_bass_kernel_spmd

    global LAST_TRACE_DIR
    kw = {}
    if _TRACE:
        import tempfile

        LAST_TRACE_DIR = tempfile.mkdtemp(prefix="bass_trace_")
        kw["tmpdir"] = LAST_TRACE_DIR
    res = run_bass_kernel_spmd(nc, in_maps, list(range(NCORES)), trace=_TRACE, **kw)
    LAST_EXEC_NS = res.exec_time_ns
    LAST_RESULTS = res
    out = np.concatenate([r["v_out"] for r in res.results], axis=0)
    return out.astype(np.float32)


# revision 30
# speedup vs baseline: 1.1141x; 1.1141x over previous
"""Trainium2 Bass kernel for AgreementRouting (dynamic routing / capsule-style).

Full-input contract: kernel(u_predict[64,2048,32,16] f32, b[2048,32] f32) -> v[64,32,16] f32.
Internally shards batch (64) across 8 NeuronCores (8 batch elems per core).

Per-core algorithm (B_loc=8, L=2048, H=32, D=16, HD=512), fp16 compute with
fp32 accumulation; batch elements processed in two resident groups of 4 with
phase-staggered emission so PE matmul bursts from different batch elements
interleave (keeps the PE HAM-warm and batches same-LUT ACT ops):
  load: u fp32 HBM -> SBUF fp16 via gpsimd casting DMA (16 tiles [128 l, 512 hd]/b)
  u^T:  PE transpose-mode matmuls (4 per PSUM bank) + DVE bitcast-u32 evac
        -> 4 tiles [128 hd, 2048 l] fp16 per b
  init: c0 = softmax(b) (shared); then per routing iteration:
    agreement: upd[l,h] = sum_hd u^T[hd,l]*V[hd,h], u^T slices as FWL fp16
               weights, block-diag V_mat [128,32] as rhs; += into b_batch fp32
    softmax:   half-width-split add/exp/reduce/recip/mult chain (ACT+DVE)
    ws:        O2[hd,h'] = sum_l u[l,hd]*c[l,h'] with u slices as weights;
               s extracted via DVE mask-multiply + inner-reduce (no PSUM round
               trip); squash factor via tiny constant matmuls (h_mat/ind_t)
               entirely in [hd-partition] layout; V_mat built by broadcast-mult
               with ind_mask
  output: final v^T [128,4] fp32 DMA'd per batch elem
"""

import sys
import os

sys.path.insert(0, "/opt/trn_rl_repo")

import numpy as np
from contextlib import ExitStack

B, L, H, D = 64, 2048, 32, 16
NCORES = 8
BLOC = B // NCORES  # 8
HD = H * D  # 512
NT = L // 128  # 16 l-chunks
NKC = HD // 128  # 4 hd chunks
NITER = 3
EPS = 1e-8

_NC_CACHE = {}
LAST_EXEC_NS = None
LAST_RESULTS = None
LAST_TRACE_DIR = None
_TRACE = False


def _consts():
    p = np.arange(128)
    j = np.arange(HD)
    # mask_hd[h', hd] = 1 iff h' == hd//16
    mask_hd = (j[None, :] // D == np.arange(H)[:, None]).astype(np.float32)
    # ind_mask[p, H*c + h] = 1 iff h == 8c + p//16   (h-index of hd = 128c + p)
    ind_mask = np.zeros((128, NKC * H), np.float32)
    for c in range(NKC):
        ind_mask[p, H * c + 8 * c + p // 16] = 1.0
    # h_mat[p, g] = 1 iff p//16 == g
    h_mat = (p[:, None] // 16 == np.arange(8)[None, :]).astype(np.float32)
    ind_t = np.ascontiguousarray(h_mat.T)  # [8, 128]
    ident16 = np.eye(128, dtype=np.float16)
    return {
        "ind_mask": ind_mask,
        "h16": h_mat,
        "it16": ind_t,
        "ident16": ident16,
    }


def _emit(ctx, tc, t_in, t_out):
    import concourse.mybir as mybir

    nc = tc.nc
    f32 = mybir.dt.float32
    f16 = mybir.dt.float16
    AF = mybir.ActivationFunctionType
    ALU = mybir.AluOpType
    AX = mybir.AxisListType

    u_ap = t_in["u"]
    b_ap = t_in["b"]
    im_ap = t_in["ind_mask"]
    h_ap = t_in["h16"]
    it_ap = t_in["it16"]
    id_ap = t_in["ident16"]
    vout_ap = t_out["v_out"]

    GRP = 4  # batch elems per resident group

    cpool = ctx.enter_context(tc.tile_pool(name="cpool", bufs=1))
    p_unat = ctx.enter_context(tc.tile_pool(name="p_unat", bufs=(GRP + 1) * NT))
    p_uT = ctx.enter_context(tc.tile_pool(name="p_uT", bufs=(GRP + 1) * NKC))
    p_bb = ctx.enter_context(tc.tile_pool(name="p_bb", bufs=BLOC))
    p_soft = ctx.enter_context(tc.tile_pool(name="p_soft", bufs=6))
    p_small = ctx.enter_context(tc.tile_pool(name="p_small", bufs=10))
    p_prod = ctx.enter_context(tc.tile_pool(name="p_prod", bufs=4))
    p_s4 = ctx.enter_context(tc.tile_pool(name="p_s4", bufs=12))
    p_sq = ctx.enter_context(tc.tile_pool(name="p_sq", bufs=24))
    p_vm = ctx.enter_context(tc.tile_pool(name="p_vm", bufs=2 * GRP * NKC))
    ps_upd = ctx.enter_context(tc.tile_pool(name="ps_upd", bufs=2, space="PSUM"))
    ps_o = ctx.enter_context(tc.tile_pool(name="ps_o", bufs=2, space="PSUM"))
    ps_t = ctx.enter_context(tc.tile_pool(name="ps_t", bufs=2, space="PSUM"))
    ps_tr = ctx.enter_context(tc.tile_pool(name="ps_tr", bufs=2, space="PSUM"))

    # ---- constants
    im_t = cpool.tile([128, NKC * H], f32, name="im_t")
    nc.sync.dma_start(im_t[:], im_ap)
    h_t = cpool.tile([128, 8], f32, name="h_t")
    nc.sync.dma_start(h_t[:], h_ap)
    it_t = cpool.tile([8, 128], f32, name="it_t")
    nc.sync.dma_start(it_t[:], it_ap)
    id_t = cpool.tile([128, 128], f16, name="id_t")
    nc.sync.dma_start(id_t[:], id_ap)
    bin_t = cpool.tile([128, NT * H], f32, name="bin_t")
    nc.sync.dma_start(
        bin_t[:].rearrange("p (t h) -> p t h", t=NT),
        b_ap.rearrange("(t p) h -> p t h", p=128),
    )

    # ---- c0 = softmax(b) over h (shared across batch; logits bounded, so no
    # max-subtraction needed)
    e0 = p_soft.tile([128, NT * H], f32, name="e0", tag="soft")
    nc.scalar.activation(e0[:], bin_t[:], AF.Exp)
    z0 = p_small.tile([128, NT], f32, name="z0", tag="small")
    nc.vector.reduce_sum(z0[:], e0[:].rearrange("p (t h) -> p t h", t=NT), AX.X)
    r0 = p_small.tile([128, NT], f32, name="r0", tag="small")
    nc.vector.reciprocal(r0[:], z0[:])
    c0 = cpool.tile([128, NT * H], f16, name="c0")
    nc.vector.tensor_tensor(
        c0[:].rearrange("p (t h) -> p t h", t=NT),
        e0[:].rearrange("p (t h) -> p t h", t=NT),
        r0[:].unsqueeze(2).broadcast_to((128, NT, H)),
        ALU.mult,
    )

    st = {}  # per-b state

    def emit_prep(b):
        nat = []
        for t in range(NT):
            s16 = p_unat.tile([128, HD], f16, name="s16", tag="unat")
            nc.gpsimd.dma_start(
                s16[:],
                u_ap[b, 128 * t : 128 * (t + 1)].rearrange("l h d -> l (h d)"),
            )
            nat.append(s16)
        st[b] = {"nat": nat}

    def emit_transpose(b):
        nat = st[b]["nat"]
        uT = []
        for k in range(NKC):
            uTk = p_uT.tile([128, L], f16, name="uTk", tag="uT")
            uT.append(uTk)
        for k in range(NKC):
            for tq in range(NT // 4):
                ptr = ps_tr.tile([128, 4 * 128], f16, name="ptr", tag="ptr", padded_shape=[128, 1024])
                for j in range(4):
                    nc.tensor.transpose(
                        ptr[:, 128 * j : 128 * (j + 1)],
                        nat[4 * tq + j][:, 128 * k : 128 * (k + 1)],
                        id_t[:],
                    )
                dst = uT[k][:, 512 * tq : 512 * (tq + 1)]
                u32 = mybir.dt.uint32
                if (k + tq) % 2 == 0:
                    nc.vector.tensor_copy(dst.bitcast(u32), ptr[:].bitcast(u32))
                else:
                    nc.scalar.copy(dst.bitcast(u32), ptr[:].bitcast(u32))
        st[b]["uT"] = uT
        bb_t = p_bb.tile([128, NT * H], f32, name="bbt", tag="bb")
        nc.vector.tensor_copy(bb_t[:], bin_t[:])
        st[b]["bb"] = bb_t

    def emit_ws(b, c_tile, last):
        """weighted-sum via u-as-weights: O2[hd, h'] = sum_l u[l, hd] c[l, h'],
        then fused mask-multiply-reduce extracts s directly into SBUF."""
        nat = st[b]["nat"]
        O2 = ps_o.tile([128, NKC * H], f32, name="O2", tag="O", padded_shape=[128, 512])
        cv = c_tile[:].rearrange("p (t h) -> p t h", t=NT)
        for k in range(NKC):
            for t in range(NT):
                nc.tensor.matmul(
                    O2[:, H * k : H * (k + 1)],
                    nat[t][:, 128 * k : 128 * (k + 1)],
                    cv[:, t, :],
                    start=(t == 0),
                    stop=(t == NT - 1),
                )
        prod = p_prod.tile([128, NKC * H], f32, name="prod", tag="prod")
        s_sb = p_s4.tile([128, NKC], f32, name="s_sb", tag="s4")
        nc.vector.tensor_tensor(prod[:], O2[:], im_t[:], ALU.mult)
        nc.vector.reduce_sum(
            s_sb[:], prod[:].rearrange("p (k h) -> p k h", k=NKC), AX.X
        )
        s2 = p_s4.tile([128, NKC], f32, name="s2", tag="s4")
        nc.scalar.square(s2[:], s_sb[:])
        # sq^T[g, c] = ||s_h||^2 for h = 8c + g
        sqT = ps_t.tile([8, NKC], f32, name="sqT", tag="pt", padded_shape=[128, 512])
        nc.tensor.matmul(sqT[:], h_t[:], s2[:], start=True, stop=True)
        st[b]["s_sb"] = s_sb
        st[b]["sqT"] = sqT
        st[b]["last"] = last

    def emit_squash(b):
        """squash factor f = sq/(1+sq)/sqrt(sq+eps); vT; V_mat (or output DMA)."""
        sqT = st[b]["sqT"]
        s_sb = st[b]["s_sb"]
        last = st[b]["last"]
        t1 = p_sq.tile([8, NKC], f32, name="t1", tag="sq")
        nc.vector.tensor_scalar_add(t1[:], sqT[:], 1.0)
        r1 = p_sq.tile([8, NKC], f32, name="r1", tag="sq")
        nc.vector.reciprocal(r1[:], t1[:])
        teps = p_sq.tile([8, NKC], f32, name="teps", tag="sq")
        nc.vector.tensor_scalar_add(teps[:], sqT[:], EPS)
        rt = p_sq.tile([8, NKC], f32, name="rt", tag="sq")
        nc.scalar.activation(rt[:], teps[:], AF.Sqrt)
        r2 = p_sq.tile([8, NKC], f32, name="r2", tag="sq")
        nc.vector.reciprocal(r2[:], rt[:])
        g1 = p_sq.tile([8, NKC], f32, name="g1", tag="sq")
        nc.vector.tensor_tensor(g1[:], sqT[:], r1[:], ALU.mult)
        fT = p_sq.tile([8, NKC], f32, name="fT", tag="sq")
        nc.vector.tensor_tensor(fT[:], g1[:], r2[:], ALU.mult)
        # expand f to hd-partition layout: fexp[p, c] = f[8c + p//16]
        fexp = ps_t.tile([128, NKC], f32, name="fexp", tag="pt", padded_shape=[128, 512])
        nc.tensor.matmul(fexp[:], it_t[:], fT[:], start=True, stop=True)
        vT = p_s4.tile([128, NKC], f32, name="vT", tag="s4")
        nc.vector.tensor_tensor(vT[:], s_sb[:], fexp[:], ALU.mult)
        if last:
            nc.sync.dma_start(
                vout_ap[b].rearrange("h d -> (h d)").rearrange("(c p) -> p c", p=128),
                vT[:],
            )
            st[b]["vms"] = None
            return
        vms = []
        for c in range(NKC):
            vm_c = p_vm.tile([128, H], f16, name="vmc", tag="vm")
            nc.vector.tensor_tensor(
                vm_c[:],
                vT[:, c : c + 1].broadcast_to((128, H)),
                im_t[:, H * c : H * (c + 1)],
                ALU.mult,
            )
            vms.append(vm_c)
        st[b]["vms"] = vms

    def emit_agree(b):
        """agreement matmuls + b_batch add + softmax -> fresh c tile."""
        uT = st[b]["uT"]
        vms = st[b]["vms"]
        bb_t = st[b]["bb"]
        upd = ps_upd.tile([128, NT * H], f32, name="upd", tag="upd", padded_shape=[128, 512])
        for t in range(NT):
            for k in range(NKC):
                nc.tensor.matmul(
                    upd[:, H * t : H * (t + 1)],
                    uT[k][:, 128 * t : 128 * (t + 1)],
                    vms[k][:],
                    start=(k == 0),
                    stop=(k == NKC - 1),
                )
        HW2 = NT * H // 2
        for hh in range(2):
            sl = slice(hh * HW2, (hh + 1) * HW2)
            nc.vector.tensor_tensor(bb_t[:, sl], bb_t[:, sl], upd[:, sl], ALU.add)

    def emit_softmax(b):
        # half-width split: pipeline the add/exp/reduce/mult chain to cut the
        # exposed latency before ws can start
        bb_t = st[b]["bb"]
        HW2 = NT * H // 2
        e = p_soft.tile([128, NT * H], f32, name="e", tag="soft")
        z = p_small.tile([128, NT], f32, name="z", tag="small")
        r = p_small.tile([128, NT], f32, name="r", tag="small")
        c_t = p_soft.tile([128, NT * H], f16, name="ct", tag="softc")
        for hh in range(2):
            sl = slice(hh * HW2, (hh + 1) * HW2)
            slz = slice(hh * NT // 2, (hh + 1) * NT // 2)
            nc.scalar.activation(e[:, sl], bb_t[:, sl], AF.Exp)
            nc.vector.reduce_sum(
                z[:, slz],
                e[:, sl].rearrange("p (t h) -> p t h", t=NT // 2),
                AX.X,
            )
            nc.vector.reciprocal(r[:, slz], z[:, slz])
            nc.vector.tensor_tensor(
                c_t[:, sl].rearrange("p (t h) -> p t h", t=NT // 2),
                e[:, sl].rearrange("p (t h) -> p t h", t=NT // 2),
                r[:, slz].unsqueeze(2).broadcast_to((128, NT // 2, H)),
                ALU.mult,
            )
        st[b]["c"] = c_t

    for g in range(BLOC // GRP):
        bs = list(range(g * GRP, (g + 1) * GRP))
        for b in bs:
            emit_prep(b)
        for b in bs:
            emit_transpose(b)
        # init weighted-sum pass with shared c0
        for b in bs:
            emit_ws(b, c0, False)
        for b in bs:
            emit_squash(b)
        for it in range(NITER):
            last = it == NITER - 1
            # staggered: alternate LDW-heavy agree bursts with MM-heavy ws bursts
            emit_agree(bs[0])
            emit_agree(bs[1])
            for j in range(GRP):
                emit_softmax(bs[j])
                if j + 2 < GRP:
                    emit_agree(bs[j + 2])
                emit_ws(bs[j], st[bs[j]]["c"], last)
            for b in bs:
                emit_squash(b)


def _get_nc():
    if "nc" in _NC_CACHE:
        return _NC_CACHE["nc"]
    from concourse import bacc
    import concourse.tile as tile
    import concourse.mybir as mybir

    f32 = mybir.dt.float32
    f16 = mybir.dt.float16
    nc = bacc.Bacc("TRN2", target_bir_lowering=False, debug=False)
    t_in = {}
    in_shapes = {
        "u": ([BLOC, L, H, D], f32),
        "b": ([L, H], f32),
        "ind_mask": ([128, NKC * H], f32),
        "h16": ([128, 8], f32),
        "it16": ([8, 128], f32),
        "ident16": ([128, 128], f16),
    }
    for name, (shape, dt_) in in_shapes.items():
        t_in[name] = nc.dram_tensor(name, shape, dt_, kind="ExternalInput").ap()
    vout = nc.dram_tensor("v_out", [BLOC, H, D], f32, kind="ExternalOutput").ap()

    with tile.TileContext(nc) as tc:
        with ExitStack() as ctx:
            _emit(ctx, tc, t_in, {"v_out": vout})
    nc.compile()
    _NC_CACHE["nc"] = nc
    return nc


def kernel(u_predict, b):
    global LAST_EXEC_NS, LAST_RESULTS
    u = np.ascontiguousarray(np.asarray(u_predict, dtype=np.float32))
    bq = np.ascontiguousarray(np.asarray(b, dtype=np.float32))
    assert u.shape == (B, L, H, D), u.shape
    assert bq.shape == (L, H), bq.shape

    nc = _get_nc()
    consts = _consts()
    in_maps = []
    for i in range(NCORES):
        m = {"u": np.ascontiguousarray(u[i * BLOC : (i + 1) * BLOC]), "b": bq}
        m.update(consts)
        in_maps.append(m)

    from concourse.bass_utils import run_bass_kernel_spmd

    global LAST_TRACE_DIR
    kw = {}
    if _TRACE:
        import tempfile

        LAST_TRACE_DIR = tempfile.mkdtemp(prefix="bass_trace_")
        kw["tmpdir"] = LAST_TRACE_DIR
    res = run_bass_kernel_spmd(nc, in_maps, list(range(NCORES)), trace=_TRACE, **kw)
    LAST_EXEC_NS = res.exec_time_ns
    LAST_RESULTS = res
    out = np.concatenate([r["v_out"] for r in res.results], axis=0)
    return out.astype(np.float32)


# revision 31
# speedup vs baseline: 1.1544x; 1.0361x over previous
"""Trainium2 Bass kernel for AgreementRouting (dynamic routing / capsule-style).

Full-input contract: kernel(u_predict[64,2048,32,16] f32, b[2048,32] f32) -> v[64,32,16] f32.
Internally shards batch (64) across 8 NeuronCores (8 batch elems per core).

Per-core algorithm (B_loc=8, L=2048, H=32, D=16, HD=512), fp16 compute with
fp32 accumulation; batch elements processed in two resident groups of 4 with
phase-staggered emission so PE matmul bursts from different batch elements
interleave (keeps the PE HAM-warm and batches same-LUT ACT ops):
  load: u fp32 HBM -> SBUF fp16 via gpsimd casting DMA (16 tiles [128 l, 512 hd]/b)
  u^T:  PE transpose-mode matmuls (4 per PSUM bank) + DVE bitcast-u32 evac
        -> 4 tiles [128 hd, 2048 l] fp16 per b
  init: c0 = softmax(b) (shared); then per routing iteration:
    agreement: upd[l,h] = sum_hd u^T[hd,l]*V[hd,h], u^T slices as FWL fp16
               weights, block-diag V_mat [128,32] as rhs; += into b_batch fp32
    softmax:   half-width-split add/exp/reduce/recip/mult chain (ACT+DVE)
    ws:        O2[hd,h'] = sum_l u[l,hd]*c[l,h'] with u slices as weights;
               s extracted via DVE mask-multiply + inner-reduce (no PSUM round
               trip); squash factor via tiny constant matmuls (h_mat/ind_t)
               entirely in [hd-partition] layout; V_mat built by broadcast-mult
               with ind_mask
  output: final v^T [128,4] fp32 DMA'd per batch elem
"""

import sys
import os

sys.path.insert(0, "/opt/trn_rl_repo")

import numpy as np
from contextlib import ExitStack

B, L, H, D = 64, 2048, 32, 16
NCORES = 8
BLOC = B // NCORES  # 8
HD = H * D  # 512
NT = L // 128  # 16 l-chunks
NKC = HD // 128  # 4 hd chunks
NITER = 3
EPS = 1e-8

_NC_CACHE = {}
LAST_EXEC_NS = None
LAST_RESULTS = None
LAST_TRACE_DIR = None
_TRACE = False


def _consts():
    p = np.arange(128)
    j = np.arange(HD)
    # mask_hd[h', hd] = 1 iff h' == hd//16
    mask_hd = (j[None, :] // D == np.arange(H)[:, None]).astype(np.float32)
    # ind_mask[p, H*c + h] = 1 iff h == 8c + p//16   (h-index of hd = 128c + p)
    ind_mask = np.zeros((128, NKC * H), np.float32)
    for c in range(NKC):
        ind_mask[p, H * c + 8 * c + p // 16] = 1.0
    # h_mat[p, g] = 1 iff p//16 == g
    h_mat = (p[:, None] // 16 == np.arange(8)[None, :]).astype(np.float32)
    ind_t = np.ascontiguousarray(h_mat.T)  # [8, 128]
    ident16 = np.eye(128, dtype=np.float16)
    return {
        "ind_mask": ind_mask,
        "h16": h_mat,
        "it16": ind_t,
        "ident16": ident16,
    }


def _emit(ctx, tc, t_in, t_out):
    import concourse.mybir as mybir

    nc = tc.nc
    f32 = mybir.dt.float32
    f16 = mybir.dt.float16
    AF = mybir.ActivationFunctionType
    ALU = mybir.AluOpType
    AX = mybir.AxisListType

    u_ap = t_in["u"]
    b_ap = t_in["b"]
    im_ap = t_in["ind_mask"]
    h_ap = t_in["h16"]
    it_ap = t_in["it16"]
    id_ap = t_in["ident16"]
    vout_ap = t_out["v_out"]

    GRP = 4  # batch elems per resident group

    cpool = ctx.enter_context(tc.tile_pool(name="cpool", bufs=1))
    p_unat = ctx.enter_context(tc.tile_pool(name="p_unat", bufs=(GRP + 1) * NT))
    p_uT = ctx.enter_context(tc.tile_pool(name="p_uT", bufs=(GRP + 1) * NKC))
    p_bb = ctx.enter_context(tc.tile_pool(name="p_bb", bufs=BLOC))
    p_soft = ctx.enter_context(tc.tile_pool(name="p_soft", bufs=6))
    p_small = ctx.enter_context(tc.tile_pool(name="p_small", bufs=10))
    p_prod = ctx.enter_context(tc.tile_pool(name="p_prod", bufs=4))
    p_s4 = ctx.enter_context(tc.tile_pool(name="p_s4", bufs=12))
    p_sq = ctx.enter_context(tc.tile_pool(name="p_sq", bufs=24))
    p_vm = ctx.enter_context(tc.tile_pool(name="p_vm", bufs=2 * GRP * NKC))
    ps_upd = ctx.enter_context(tc.tile_pool(name="ps_upd", bufs=2, space="PSUM"))
    ps_o = ctx.enter_context(tc.tile_pool(name="ps_o", bufs=2, space="PSUM"))
    ps_t = ctx.enter_context(tc.tile_pool(name="ps_t", bufs=2, space="PSUM"))
    ps_tr = ctx.enter_context(tc.tile_pool(name="ps_tr", bufs=2, space="PSUM"))

    # ---- constants
    im_t = cpool.tile([128, NKC * H], f32, name="im_t")
    nc.sync.dma_start(im_t[:], im_ap)
    h_t = cpool.tile([128, 8], f32, name="h_t")
    nc.sync.dma_start(h_t[:], h_ap)
    it_t = cpool.tile([8, 128], f32, name="it_t")
    nc.sync.dma_start(it_t[:], it_ap)
    id_t = cpool.tile([128, 128], f16, name="id_t")
    nc.sync.dma_start(id_t[:], id_ap)
    bin_t = cpool.tile([128, NT * H], f32, name="bin_t")
    nc.sync.dma_start(
        bin_t[:].rearrange("p (t h) -> p t h", t=NT),
        b_ap.rearrange("(t p) h -> p t h", p=128),
    )

    # ---- c0 = softmax(b) over h (shared across batch; logits bounded, so no
    # max-subtraction needed)
    e0 = p_soft.tile([128, NT * H], f32, name="e0", tag="soft")
    nc.scalar.activation(e0[:], bin_t[:], AF.Exp)
    z0 = p_small.tile([128, NT], f32, name="z0", tag="small")
    nc.vector.reduce_sum(z0[:], e0[:].rearrange("p (t h) -> p t h", t=NT), AX.X)
    r0 = p_small.tile([128, NT], f32, name="r0", tag="small")
    nc.vector.reciprocal(r0[:], z0[:])
    c0 = cpool.tile([128, NT * H], f16, name="c0")
    nc.vector.tensor_tensor(
        c0[:].rearrange("p (t h) -> p t h", t=NT),
        e0[:].rearrange("p (t h) -> p t h", t=NT),
        r0[:].unsqueeze(2).broadcast_to((128, NT, H)),
        ALU.mult,
    )

    st = {}  # per-b state

    def emit_prep(b):
        nat = []
        for t in range(NT):
            s16 = p_unat.tile([128, HD], f16, name="s16", tag="unat")
            nc.gpsimd.dma_start(
                s16[:],
                u_ap[b, 128 * t : 128 * (t + 1)].rearrange("l h d -> l (h d)"),
            )
            nat.append(s16)
        st[b] = {"nat": nat}

    def emit_transpose(b):
        nat = st[b]["nat"]
        uT = []
        for k in range(NKC):
            uTk = p_uT.tile([128, L], f16, name="uTk", tag="uT")
            uT.append(uTk)
        for k in range(NKC):
            for tq in range(NT // 4):
                ptr = ps_tr.tile([128, 4 * 128], f16, name="ptr", tag="ptr", padded_shape=[128, 1024])
                for j in range(4):
                    nc.tensor.transpose(
                        ptr[:, 128 * j : 128 * (j + 1)],
                        nat[4 * tq + j][:, 128 * k : 128 * (k + 1)],
                        id_t[:],
                    )
                dst = uT[k][:, 512 * tq : 512 * (tq + 1)]
                u32 = mybir.dt.uint32
                nc.vector.tensor_copy(dst.bitcast(u32), ptr[:].bitcast(u32))
        st[b]["uT"] = uT
        bb_t = p_bb.tile([128, NT * H], f32, name="bbt", tag="bb")
        nc.vector.tensor_copy(bb_t[:], bin_t[:])
        st[b]["bb"] = bb_t

    def emit_ws(b, c_tile, last):
        """weighted-sum via u-as-weights: O2[hd, h'] = sum_l u[l, hd] c[l, h'],
        then fused mask-multiply-reduce extracts s directly into SBUF."""
        nat = st[b]["nat"]
        O2 = ps_o.tile([128, NKC * H], f32, name="O2", tag="O", padded_shape=[128, 512])
        cv = c_tile[:].rearrange("p (t h) -> p t h", t=NT)
        for k in range(NKC):
            for t in range(NT):
                nc.tensor.matmul(
                    O2[:, H * k : H * (k + 1)],
                    nat[t][:, 128 * k : 128 * (k + 1)],
                    cv[:, t, :],
                    start=(t == 0),
                    stop=(t == NT - 1),
                )
        prod = p_prod.tile([128, NKC * H], f32, name="prod", tag="prod")
        s_sb = p_s4.tile([128, NKC], f32, name="s_sb", tag="s4")
        nc.vector.tensor_tensor(prod[:], O2[:], im_t[:], ALU.mult)
        nc.vector.reduce_sum(
            s_sb[:], prod[:].rearrange("p (k h) -> p k h", k=NKC), AX.X
        )
        s2 = p_s4.tile([128, NKC], f32, name="s2", tag="s4")
        nc.scalar.square(s2[:], s_sb[:])
        # sq^T[g, c] = ||s_h||^2 for h = 8c + g
        sqT = ps_t.tile([8, NKC], f32, name="sqT", tag="pt", padded_shape=[128, 512])
        nc.tensor.matmul(sqT[:], h_t[:], s2[:], start=True, stop=True)
        st[b]["s_sb"] = s_sb
        st[b]["sqT"] = sqT
        st[b]["last"] = last

    def emit_squash(b):
        """squash factor f = sq/(1+sq)/sqrt(sq+eps); vT; V_mat (or output DMA)."""
        sqT = st[b]["sqT"]
        s_sb = st[b]["s_sb"]
        last = st[b]["last"]
        t1 = p_sq.tile([8, NKC], f32, name="t1", tag="sq")
        nc.vector.tensor_scalar_add(t1[:], sqT[:], 1.0)
        r1 = p_sq.tile([8, NKC], f32, name="r1", tag="sq")
        nc.vector.reciprocal(r1[:], t1[:])
        teps = p_sq.tile([8, NKC], f32, name="teps", tag="sq")
        nc.vector.tensor_scalar_add(teps[:], sqT[:], EPS)
        rt = p_sq.tile([8, NKC], f32, name="rt", tag="sq")
        nc.scalar.activation(rt[:], teps[:], AF.Sqrt)
        r2 = p_sq.tile([8, NKC], f32, name="r2", tag="sq")
        nc.vector.reciprocal(r2[:], rt[:])
        g1 = p_sq.tile([8, NKC], f32, name="g1", tag="sq")
        nc.vector.tensor_tensor(g1[:], sqT[:], r1[:], ALU.mult)
        fT = p_sq.tile([8, NKC], f32, name="fT", tag="sq")
        nc.vector.tensor_tensor(fT[:], g1[:], r2[:], ALU.mult)
        # expand f to hd-partition layout: fexp[p, c] = f[8c + p//16]
        fexp = ps_t.tile([128, NKC], f32, name="fexp", tag="pt", padded_shape=[128, 512])
        nc.tensor.matmul(fexp[:], it_t[:], fT[:], start=True, stop=True)
        vT = p_s4.tile([128, NKC], f32, name="vT", tag="s4")
        nc.vector.tensor_tensor(vT[:], s_sb[:], fexp[:], ALU.mult)
        if last:
            nc.sync.dma_start(
                vout_ap[b].rearrange("h d -> (h d)").rearrange("(c p) -> p c", p=128),
                vT[:],
            )
            st[b]["vms"] = None
            return
        vms = []
        for c in range(NKC):
            vm_c = p_vm.tile([128, H], f16, name="vmc", tag="vm")
            nc.vector.tensor_tensor(
                vm_c[:],
                vT[:, c : c + 1].broadcast_to((128, H)),
                im_t[:, H * c : H * (c + 1)],
                ALU.mult,
            )
            vms.append(vm_c)
        st[b]["vms"] = vms

    def emit_agree(b):
        """agreement matmuls + b_batch add + softmax -> fresh c tile."""
        uT = st[b]["uT"]
        vms = st[b]["vms"]
        bb_t = st[b]["bb"]
        upd = ps_upd.tile([128, NT * H], f32, name="upd", tag="upd", padded_shape=[128, 512])
        for t in range(NT):
            for k in range(NKC):
                nc.tensor.matmul(
                    upd[:, H * t : H * (t + 1)],
                    uT[k][:, 128 * t : 128 * (t + 1)],
                    vms[k][:],
                    start=(k == 0),
                    stop=(k == NKC - 1),
                )
        HW2 = NT * H // 2
        for hh in range(2):
            sl = slice(hh * HW2, (hh + 1) * HW2)
            nc.vector.tensor_tensor(bb_t[:, sl], bb_t[:, sl], upd[:, sl], ALU.add)

    def emit_softmax(b):
        # half-width split: pipeline the add/exp/reduce/mult chain to cut the
        # exposed latency before ws can start
        bb_t = st[b]["bb"]
        HW2 = NT * H // 2
        e = p_soft.tile([128, NT * H], f32, name="e", tag="soft")
        z = p_small.tile([128, NT], f32, name="z", tag="small")
        r = p_small.tile([128, NT], f32, name="r", tag="small")
        c_t = p_soft.tile([128, NT * H], f16, name="ct", tag="softc")
        for hh in range(2):
            sl = slice(hh * HW2, (hh + 1) * HW2)
            slz = slice(hh * NT // 2, (hh + 1) * NT // 2)
            nc.scalar.activation(e[:, sl], bb_t[:, sl], AF.Exp)
            nc.vector.reduce_sum(
                z[:, slz],
                e[:, sl].rearrange("p (t h) -> p t h", t=NT // 2),
                AX.X,
            )
            nc.vector.reciprocal(r[:, slz], z[:, slz])
            nc.vector.tensor_tensor(
                c_t[:, sl].rearrange("p (t h) -> p t h", t=NT // 2),
                e[:, sl].rearrange("p (t h) -> p t h", t=NT // 2),
                r[:, slz].unsqueeze(2).broadcast_to((128, NT // 2, H)),
                ALU.mult,
            )
        st[b]["c"] = c_t

    for g in range(BLOC // GRP):
        bs = list(range(g * GRP, (g + 1) * GRP))
        for b in bs:
            emit_prep(b)
        for b in bs:
            emit_transpose(b)
        # init weighted-sum pass with shared c0
        for b in bs:
            emit_ws(b, c0, False)
        for b in bs:
            emit_squash(b)
        for it in range(NITER):
            last = it == NITER - 1
            # staggered: alternate LDW-heavy agree bursts with MM-heavy ws bursts
            emit_agree(bs[0])
            emit_agree(bs[1])
            for j in range(GRP):
                emit_softmax(bs[j])
                if j + 2 < GRP:
                    emit_agree(bs[j + 2])
                emit_ws(bs[j], st[bs[j]]["c"], last)
            for b in bs:
                emit_squash(b)


def _get_nc():
    if "nc" in _NC_CACHE:
        return _NC_CACHE["nc"]
    from concourse import bacc
    import concourse.tile as tile
    import concourse.mybir as mybir

    f32 = mybir.dt.float32
    f16 = mybir.dt.float16
    nc = bacc.Bacc("TRN2", target_bir_lowering=False, debug=False)
    t_in = {}
    in_shapes = {
        "u": ([BLOC, L, H, D], f32),
        "b": ([L, H], f32),
        "ind_mask": ([128, NKC * H], f32),
        "h16": ([128, 8], f32),
        "it16": ([8, 128], f32),
        "ident16": ([128, 128], f16),
    }
    for name, (shape, dt_) in in_shapes.items():
        t_in[name] = nc.dram_tensor(name, shape, dt_, kind="ExternalInput").ap()
    vout = nc.dram_tensor("v_out", [BLOC, H, D], f32, kind="ExternalOutput").ap()

    with tile.TileContext(nc) as tc:
        with ExitStack() as ctx:
            _emit(ctx, tc, t_in, {"v_out": vout})
    nc.compile()
    _NC_CACHE["nc"] = nc
    return nc


def kernel(u_predict, b):
    global LAST_EXEC_NS, LAST_RESULTS
    u = np.ascontiguousarray(np.asarray(u_predict, dtype=np.float32))
    bq = np.ascontiguousarray(np.asarray(b, dtype=np.float32))
    assert u.shape == (B, L, H, D), u.shape
    assert bq.shape == (L, H), bq.shape

    nc = _get_nc()
    consts = _consts()
    in_maps = []
    for i in range(NCORES):
        m = {"u": np.ascontiguousarray(u[i * BLOC : (i + 1) * BLOC]), "b": bq}
        m.update(consts)
        in_maps.append(m)

    from concourse.bass_utils import run_bass_kernel_spmd

    global LAST_TRACE_DIR
    kw = {}
    if _TRACE:
        import tempfile

        LAST_TRACE_DIR = tempfile.mkdtemp(prefix="bass_trace_")
        kw["tmpdir"] = LAST_TRACE_DIR
    res = run_bass_kernel_spmd(nc, in_maps, list(range(NCORES)), trace=_TRACE, **kw)
    LAST_EXEC_NS = res.exec_time_ns
    LAST_RESULTS = res
    out = np.concatenate([r["v_out"] for r in res.results], axis=0)
    return out.astype(np.float32)
